# revision 1
# baseline (speedup 1.0000x reference)
"""3x3 NMS (maxpool + threshold + border) kernel for Trainium2, 8 NeuronCores.

Strategy:
  - Pure data parallel: 16 images -> 2 images per core on 8 cores.
  - Host zero-pads each image to H+2 rows so the kernel can load, per
    partition, R+2 consecutive rows (R=12 core rows + 1 halo row each
    side) with a single overlapping strided DMA. Partition p of a tile
    holds padded rows p*R .. p*R+R+1 (= image rows p*R-1 .. p*R+R).
  - The image is split into NT column tiles (2-col halos) to fit SBUF.
  - Per tile, 3 vector-engine ops (all exact max/compare, no arithmetic):
      1. v1 = max(x_up, x_dn)        (stock tensor_tensor)
      2. vm = max(v1, x)             (vertical 3-window max)
      3. mask[c] = (x[c] >= max(vm[c-1], vm[c], vm[c+1], 0.6)) as u8
         -- a hand-built DVE uop (ANT_SLIDE_MAX3_GE) that computes a
         horizontal sliding-window max fused with the threshold clamp
         and the compare in ONE 1-elem/cycle pass, using delay-chain
         captures from CURR_ALU_OUT for the previous-element taps.
    x >= max(window incl. center, thr) is exactly
    (x == maxpool3x3(x)) & (x >= thr): bit-exact, no FP hazards.
  - Host: zero 10px border, np.nonzero -> (y, x) rows, exactly matching
    jnp.nonzero order (batch-major, then row, then col).
"""

import os
import sys

sys.path.insert(0, "/opt/trn_rl_repo")

import numpy as np

B, C, H, W = 16, 1, 1536, 1536
HP = H + 2                    # padded rows
N_CORES = 8
B_PER = B // N_CORES          # images per core
R = 12                        # rows per partition (128 * 12 = 1536)
NT = 4                        # column tiles per image
V = W // NT                   # valid (output) columns per tile
PAD = 2                       # column halo on each side
REP_THR = 0.6

_CACHE = {}
LAST_RESULTS = None


def _build_program():
    import concourse.bass as bass
    import concourse.bacc as bacc
    import concourse.mybir as mybir
    from concourse.tile import TileContext

    f32 = mybir.dt.float32
    u8 = mybir.dt.uint8
    MAX = mybir.AluOpType.max
    GE = mybir.AluOpType.is_ge

    from concourse.dve_ops import DveOp, OPS, _COMPILE_CACHE
    from concourse.dve_spec import Spec, Src0, Src1, C0, maxx, sq, lower, AluOp
    from concourse.dve_uop import (
        DveOpSpec, UopConfig, InpSel, OutSel, OutPath, AluInp, DelayInp,
    )
    from concourse.dve_ops import get_dve_sub_opcode


    def _mk_slide_uop(base_uop, with_cmp):
        u = base_uop  # copy of a lowered stock uop: keeps FSM/trigger/ctrl fields
        # input lanes: lane k surfaces as PREV_DELAY_{k-1} at block 0
        for i in range(len(u.inp)):
            u.inp_enable[i] = 0
        u.enable_input(InpSel.SRC_0, 1)
        if with_cmp:
            u.enable_input(InpSel.CONST_0, 2)
            u.enable_input(InpSel.SRC_1, 3)
        for p in u.out_enable:
            u.out_enable[p] = 0
        u.enable_output(OutSel.ALU_OUT, OutPath.WR0_LO)
        u.require_inp0 = 1
        u.require_inp1 = 1 if with_cmp else 0

        dp = u.datapath_config
        for b in dp:
            b.op = AluOp.BYPASS
            b.alu_src0 = AluInp.PREV_ALU_OUT
            b.alu_src1 = AluInp.PREV_ALU_OUT
            b.alu_out_enable = 1
            b.swap_enable = 0
            b.alu_out_a_enable = 0
            b.alu_out_b_enable = 0
            for c in range(len(b.delay)):
                b.delay[c] = DelayInp.PREV_ALU_OUT
                b.delay_enable[c] = 0

        if not with_cmp:
            # blk0: s0 = x(i); chain1 <- x(i-1)
            dp[0].enable_alu(AluOp.BYPASS, AluInp.PREV_DELAY_0)
            dp[0].enable_delay_from_src(DelayInp.CURR_ALU_OUT, 1)
            # blk1: m1 = max(x(i), x(i-1)); chain2 <- m1(i-1)
            dp[1].enable_alu(AluOp.MAX, AluInp.PREV_ALU_OUT, AluInp.PREV_DELAY_1)
            dp[1].enable_delay_from_src(DelayInp.CURR_ALU_OUT, 2)
            # blk2: M = max(m1(i), m1(i-1)) = max(x(i-2..i))
            dp[2].enable_alu(AluOp.MAX, AluInp.PREV_ALU_OUT, AluInp.PREV_DELAY_2)
        else:
            # chains: 1 = C0, 2 = Src1 (spatial), 3/4 = temporal captures
            dp[0].enable_alu(AluOp.BYPASS, AluInp.PREV_DELAY_0)
            dp[0].pass_through_delay(1, 2)
            dp[0].enable_delay_from_src(DelayInp.CURR_ALU_OUT, 3)
            dp[1].enable_alu(AluOp.MAX, AluInp.PREV_ALU_OUT, AluInp.PREV_DELAY_3)
            dp[1].pass_through_delay(1, 2)
            dp[1].enable_delay_from_src(DelayInp.CURR_ALU_OUT, 4)
            dp[2].enable_alu(AluOp.MAX, AluInp.PREV_ALU_OUT, AluInp.PREV_DELAY_4)
            dp[2].pass_through_delay(1, 2)
            # blk3: clamp with C0
            dp[3].enable_alu(AluOp.MAX, AluInp.PREV_ALU_OUT, AluInp.PREV_DELAY_1)
            dp[3].pass_through_delay(2)
            # blk4: out = (Mc <= Src1)  i.e. Src1 >= window max
            dp[4].enable_alu(AluOp.IS_LE, AluInp.PREV_ALU_OUT, AluInp.PREV_DELAY_2)
        return u


    _READY = {}


    def make_ops(ver="v3"):
        if _READY:
            return _READY["m3"], _READY["m3ge"]
        base1 = lower(Spec(body=sq(Src0)), ver=ver)
        base2 = lower(Spec(body=maxx(maxx(Src0, C0), Src1)), ver=ver)
        assert len(base1) == 1 and len(base2) == 1, (len(base1), len(base2))

        m3_spec = Spec(body=sq(Src0))        # dummy; never lowered (cache hit)
        m3ge_spec = Spec(body=maxx(maxx(Src0, C0), Src1))

        M3 = DveOp("ANT_SLIDE_MAX3", m3_spec, subdim=False, uops_sha={})
        M3GE = DveOp("ANT_SLIDE_MAX3_GE", m3ge_spec, subdim=False, uops_sha={})
        import concourse.dve_ops as dmod
        OPS.append(M3)
        OPS.append(M3GE)
        for i, op in enumerate(OPS):
            dmod._SUB_OPCODE_FOR_NAME[op.name] = dmod._CUSTOM_DVE_ROW_BASE + i
        dmod.CUSTOM_DVE_SPECS[M3.name] = M3.spec
        dmod.CUSTOM_DVE_SPECS[M3GE.name] = M3GE.spec

        u3 = _mk_slide_uop(base1[0], with_cmp=False)
        u3ge = _mk_slide_uop(base2[0], with_cmp=True)

        _COMPILE_CACHE[("ANT_SLIDE_MAX3", ver)] = DveOpSpec(
            name="ANT_SLIDE_MAX3", opcode=get_dve_sub_opcode("ANT_SLIDE_MAX3"),
            uops=[u3], rd1_en=False)
        _COMPILE_CACHE[("ANT_SLIDE_MAX3_GE", ver)] = DveOpSpec(
            name="ANT_SLIDE_MAX3_GE", opcode=get_dve_sub_opcode("ANT_SLIDE_MAX3_GE"),
            uops=[u3ge], rd1_en=True)
        _READY["m3"] = M3
        _READY["m3ge"] = M3GE
        return M3, M3GE

    M3, M3GE = make_ops()

    nc = bacc.Bacc()
    x_in = nc.declare_dram_parameter("x", [B_PER, HP, W], f32, isOutput=False)
    m_out = nc.declare_dram_parameter("mask", [B_PER, H, W], u8, isOutput=True)

    with TileContext(nc) as tc:
        with tc.tile_pool(name="pool", bufs=1) as pool:
            for img in range(B_PER):
                mi = m_out[img].rearrange("(p r) c -> p r c", r=R)
                for t in range(NT):
                    cs = max(t * V - PAD, 0)
                    ce = min(t * V + V + PAD, W)
                    WT = ce - cs
                    a = t * V - cs  # local col offset of the valid range

                    # overlapping strided view: partition p, row slot j,
                    # col c  ->  x[img, p*R + j, cs + c]
                    xi = bass.AP(x_in, img * HP * W + cs,
                                 [[R * W, 128], [W, R + 2], [1, WT]])

                    X = pool.tile([128, R + 2, WT], f32, tag="X", bufs=2,
                                  name=f"X_{img}_{t}")
                    VM = pool.tile([128, R + 2, WT], f32, tag="VM", bufs=1,
                                   name=f"VM_{img}_{t}")
                    MW = V if t == 0 else V + 2
                    MSK = pool.tile([128, R, V + 2], u8, tag="MSK", bufs=2,
                                    name=f"MSK_{img}_{t}")

                    nc.sync.dma_start(out=X[:, :, :], in_=xi)

                    # Pass 1: vertical 3-window max via two row-major TTs
                    # (a column-major sliding stream pays ~9 cycles per
                    # 14-element inner run — slower than two stock passes).
                    # VM slot j+2 = vmax centered image row p*R+j, matching
                    # what pass 2 expects.
                    nc.vector.tensor_tensor(
                        VM[:, 2:R + 2, :], X[:, 0:R, :], X[:, 2:R + 2, :], MAX)
                    nc.vector.tensor_tensor(
                        VM[:, 2:R + 2, :], VM[:, 2:R + 2, :],
                        X[:, 1:R + 1, :], MAX)

                    # Pass 2: horizontal sliding max3 over vm, fused with the
                    # 0.6 clamp and the (x >= M) compare, row-major streams.
                    # Junk at the first 2 cols of each row lands in discarded
                    # scratch cols (or border cols 0,1 for the first tile).
                    if t == 0:
                        # out col k = mask col k; window centered k
                        nc.vector._custom_dve(
                            M3GE,
                            out=MSK[:, :, 0:V],
                            in0=VM[:, 2:R + 2, 1:V + 1],
                            in1=X[:, 1:R + 1, 0:V],
                            s0=REP_THR)
                        nc.sync.dma_start(out=mi[:, :, 0:V],
                                          in_=MSK[:, :, 0:V])
                    else:
                        # out col k = mask col t*V-2+k; valid k in [2, V+2).
                        # On the last tile the final column's window would
                        # read past the image edge: shorten the stream by one
                        # and leave mask col W-1 (border, host-zeroed) junk.
                        SL = V + 2 if t < NT - 1 else V + 1
                        nc.vector._custom_dve(
                            M3GE,
                            out=MSK[:, :, 0:SL],
                            in0=VM[:, 2:R + 2, a - 1:a - 1 + SL],
                            in1=X[:, 1:R + 1, a - 2:a - 2 + SL],
                            s0=REP_THR)
                        nc.sync.dma_start(out=mi[:, :, t * V:(t + 1) * V],
                                          in_=MSK[:, :, 2:V + 2])
    nc.finalize()
    return nc


def _get_program():
    if "nc" not in _CACHE:
        _CACHE["nc"] = _build_program()
    return _CACHE["nc"]


def kernel(repeatability):
    global LAST_RESULTS
    from concourse.bass_utils import run_bass_kernel_spmd

    x = np.asarray(repeatability, dtype=np.float32).reshape(B, H, W)
    xp = np.zeros((B, HP, W), dtype=np.float32)
    xp[:, 1:H + 1, :] = x
    per_core = xp.reshape(N_CORES, B_PER, HP, W)
    in_maps = [{"x": np.ascontiguousarray(per_core[i])} for i in range(N_CORES)]

    nc = _get_program()
    res = run_bass_kernel_spmd(nc, in_maps, list(range(N_CORES)),
                               trace=bool(os.environ.get("NMS_TRACE")))
    LAST_RESULTS = res

    masks = np.stack([res.results[i]["mask"] for i in range(N_CORES)])
    mask_full = masks.reshape(B, C, H, W) != 0
    mask_full[:, :, :10, :] = False
    mask_full[:, :, -10:, :] = False
    mask_full[:, :, :, :10] = False
    mask_full[:, :, :, -10:] = False
    _, _, ys, xs = np.nonzero(mask_full)
    return np.stack([ys, xs]).astype(np.int32)



# revision 3
# speedup vs baseline: 1.2845x; 1.2845x over previous
"""3x3 NMS (maxpool + threshold + border) kernel for Trainium2, 8 NeuronCores.

Strategy:
  - Pure data parallel: 16 images -> 2 images per core on 8 cores.
  - Host zero-pads each image to H+2 rows so the kernel can load, per
    partition, R+2 consecutive rows (R=12 core rows + 1 halo row each
    side) with a single overlapping strided DMA. Partition p of a tile
    holds padded rows p*R .. p*R+R+1 (= image rows p*R-1 .. p*R+R).
  - The image is split into NT column tiles (2-col halos) to fit SBUF.
  - Work is split across two engines per tile:
      1. Pool (gpsimd): v1 = max(x_up, x_dn)   (vertical pair max)
      2. DVE: one fused custom uop (ANT_NMS_FUSED) that per stream
         element computes vm = max(v1, x), the horizontal sliding
         3-window max of vm, the 0.6 threshold clamp, and the compare
         mask[c] = (x[c] >= max(vm[c-1], vm[c], vm[c+1], 0.6)) in a
         single 1-elem/cycle pass, using delay-chain captures for the
         previous-element taps (including a 1-delayed x for the center
         compare).
    x >= max(window incl. center, thr) is exactly
    (x == maxpool3x3(x)) & (x >= thr): bit-exact, no FP hazards.
  - Host: zero 10px border, np.nonzero -> (y, x) rows, exactly matching
    jnp.nonzero order (batch-major, then row, then col).
"""

import os
import sys

sys.path.insert(0, "/opt/trn_rl_repo")

import numpy as np

B, C, H, W = 16, 1, 1536, 1536
HP = H + 2                    # padded rows
N_CORES = 8
B_PER = B // N_CORES          # images per core
R = 12                        # rows per partition (128 * 12 = 1536)
NT = 4                        # column tiles per image
V = W // NT                   # valid (output) columns per tile
PAD = 2                       # column halo on each side
REP_THR = 0.6

_CACHE = {}
LAST_RESULTS = None


def _build_program():
    import concourse.bass as bass
    import concourse.bacc as bacc
    import concourse.mybir as mybir
    from concourse.tile import TileContext

    f32 = mybir.dt.float32
    u8 = mybir.dt.uint8
    MAX = mybir.AluOpType.max

    from concourse.dve_ops import DveOp, OPS, _COMPILE_CACHE
    from concourse.dve_spec import Spec, Src0, Src1, C0, maxx, lower
    from concourse.dve_uop import (
        DveOpSpec, InpSel, OutSel, OutPath, AluInp, DelayInp, AluOp,
    )
    from concourse.dve_ops import get_dve_sub_opcode


    def _mk_fused_uop(base_uop):
        """One fused NMS pass. Stream pos i carries v1[i] (src0) and x[i]
        (src1); output at pos i is mask for the column one behind:
        out(i) = (x(i-1) >= max(0.6, vm(i-2), vm(i-1), vm(i))) with
        vm(j) = max(v1(j), x(j)).

        Delay chains (v3 has 6): 0 = v1 in, 1 = C0, 2 = x in,
        3 = x delayed one element, 4 = vm(i-1) tap, 5 = m1(i-1) tap.
        """
        u = base_uop  # copy of a lowered stock uop: keeps FSM/trigger/ctrl
        for i in range(len(u.inp)):
            u.inp_enable[i] = 0
        u.enable_input(InpSel.SRC_0, 1)
        u.enable_input(InpSel.CONST_0, 2)
        u.enable_input(InpSel.SRC_1, 3)
        for p in u.out_enable:
            u.out_enable[p] = 0
        u.enable_output(OutSel.ALU_OUT, OutPath.WR0_LO)
        u.require_inp0 = 1
        u.require_inp1 = 1

        dp = u.datapath_config
        for b in dp:
            b.op = AluOp.BYPASS
            b.alu_src0 = AluInp.PREV_ALU_OUT
            b.alu_src1 = AluInp.PREV_ALU_OUT
            b.alu_out_enable = 1
            b.swap_enable = 0
            b.alu_out_a_enable = 0
            b.alu_out_b_enable = 0
            for c in range(len(b.delay)):
                b.delay[c] = DelayInp.PREV_ALU_OUT
                b.delay_enable[c] = 0

        # blk0: ALU = bypass(x); chain3 <- x (reads as x(i-1) downstream);
        #       carry v1 (0), C0 (1), x (2) onward
        dp[0].enable_alu(AluOp.BYPASS, AluInp.PREV_DELAY_2)
        dp[0].pass_through_delay(0, 1, 2)
        dp[0].enable_delay_from_src(DelayInp.CURR_ALU_OUT, 3)
        # blk1: vm = max(v1(i), x(i)); chain4 <- vm (reads as vm(i-1))
        dp[1].enable_alu(AluOp.MAX, AluInp.PREV_DELAY_0, AluInp.PREV_DELAY_2)
        dp[1].pass_through_delay(1, 3)
        dp[1].enable_delay_from_src(DelayInp.CURR_ALU_OUT, 4)
        # blk2: m1 = max(vm(i), vm(i-1)); chain5 <- m1 (reads as m1(i-1))
        dp[2].enable_alu(AluOp.MAX, AluInp.PREV_ALU_OUT, AluInp.PREV_DELAY_4)
        dp[2].pass_through_delay(1, 3)
        dp[2].enable_delay_from_src(DelayInp.CURR_ALU_OUT, 5)
        # blk3: M = max(m1(i), m1(i-1)) = max(vm(i-2..i))
        dp[3].enable_alu(AluOp.MAX, AluInp.PREV_ALU_OUT, AluInp.PREV_DELAY_5)
        dp[3].pass_through_delay(1, 3)
        # blk4: clamp with C0
        dp[4].enable_alu(AluOp.MAX, AluInp.PREV_ALU_OUT, AluInp.PREV_DELAY_1)
        dp[4].pass_through_delay(3)
        # blk5: out = (Mc <= x(i-1))  i.e. x(i-1) >= window max
        dp[5].enable_alu(AluOp.IS_LE, AluInp.PREV_ALU_OUT, AluInp.PREV_DELAY_3)
        return u


    _READY = {}


    def make_ops(ver="v3"):
        if _READY:
            return _READY["fused"]
        base = lower(Spec(body=maxx(maxx(Src0, C0), Src1)), ver=ver)
        assert len(base) == 1, len(base)

        fused_spec = Spec(body=maxx(maxx(Src0, C0), Src1))  # dummy; cache hit

        FUSED = DveOp("ANT_NMS_FUSED", fused_spec, subdim=False, uops_sha={})
        import concourse.dve_ops as dmod
        OPS.append(FUSED)
        for i, op in enumerate(OPS):
            dmod._SUB_OPCODE_FOR_NAME[op.name] = dmod._CUSTOM_DVE_ROW_BASE + i
        dmod.CUSTOM_DVE_SPECS[FUSED.name] = FUSED.spec

        uf = _mk_fused_uop(base[0])

        _COMPILE_CACHE[("ANT_NMS_FUSED", ver)] = DveOpSpec(
            name="ANT_NMS_FUSED", opcode=get_dve_sub_opcode("ANT_NMS_FUSED"),
            uops=[uf], rd1_en=True)
        _READY["fused"] = FUSED
        return FUSED

    FUSED = make_ops()

    nc = bacc.Bacc()
    x_in = nc.declare_dram_parameter("x", [B_PER, HP, W], f32, isOutput=False)
    m_out = nc.declare_dram_parameter("mask", [B_PER, H, W], u8, isOutput=True)

    with TileContext(nc) as tc:
        with tc.tile_pool(name="pool", bufs=1) as pool:
            for img in range(B_PER):
                mi = m_out[img].rearrange("(p r) c -> p r c", r=R)
                for t in range(NT):
                    cs = max(t * V - PAD, 0)
                    ce = min(t * V + V + PAD, W)
                    WT = ce - cs
                    a = t * V - cs  # local col offset of the valid range

                    # overlapping strided view: partition p, row slot j,
                    # col c  ->  x[img, p*R + j, cs + c]
                    xi = bass.AP(x_in, img * HP * W + cs,
                                 [[R * W, 128], [W, R + 2], [1, WT]])

                    X = pool.tile([128, R + 2, WT], f32, tag="X", bufs=3,
                                  name=f"X_{img}_{t}")
                    V1 = pool.tile([128, R, WT], f32, tag="V1", bufs=2,
                                   name=f"V1_{img}_{t}")
                    MSK = pool.tile([128, R, V + 2], u8, tag="MSK", bufs=2,
                                    name=f"MSK_{img}_{t}")

                    nc.sync.dma_start(out=X[:, :, :], in_=xi)

                    # Vertical pair max of the two outer rows. (The gpsimd
                    # Pool engine cannot run TT max in this toolchain: walrus
                    # codegen only accepts Add/Multiply there.)
                    nc.vector.tensor_tensor(
                        V1[:, :, :], X[:, 0:R, :], X[:, 2:R + 2, :], MAX)

                    # DVE: fused merge + horizontal sliding max3 + clamp +
                    # compare, row-major streams. Junk in the first 2 cols of
                    # each row lands in discarded scratch cols (or border
                    # cols 0,1 for the first tile).
                    if t == 0:
                        # out col k = mask col k; window centered k
                        nc.vector._custom_dve(
                            FUSED,
                            out=MSK[:, :, 0:V],
                            in0=V1[:, :, 1:V + 1],
                            in1=X[:, 1:R + 1, 1:V + 1],
                            s0=REP_THR)
                        nc.sync.dma_start(out=mi[:, :, 0:V],
                                          in_=MSK[:, :, 0:V])
                    else:
                        # out col k = mask col t*V-2+k; valid k in [2, V+2).
                        # On the last tile the final column's window would
                        # read past the image edge: shorten the stream by one
                        # and leave mask col W-1 (border, host-zeroed) junk.
                        SL = V + 2 if t < NT - 1 else V + 1
                        nc.vector._custom_dve(
                            FUSED,
                            out=MSK[:, :, 0:SL],
                            in0=V1[:, :, a - 1:a - 1 + SL],
                            in1=X[:, 1:R + 1, a - 1:a - 1 + SL],
                            s0=REP_THR)
                        nc.sync.dma_start(out=mi[:, :, t * V:(t + 1) * V],
                                          in_=MSK[:, :, 2:V + 2])
    nc.finalize()
    return nc


def _get_program():
    if "nc" not in _CACHE:
        _CACHE["nc"] = _build_program()
    return _CACHE["nc"]


def kernel(repeatability):
    global LAST_RESULTS
    from concourse.bass_utils import run_bass_kernel_spmd

    x = np.asarray(repeatability, dtype=np.float32).reshape(B, H, W)
    xp = np.zeros((B, HP, W), dtype=np.float32)
    xp[:, 1:H + 1, :] = x
    per_core = xp.reshape(N_CORES, B_PER, HP, W)
    in_maps = [{"x": np.ascontiguousarray(per_core[i])} for i in range(N_CORES)]

    nc = _get_program()
    res = run_bass_kernel_spmd(nc, in_maps, list(range(N_CORES)),
                               trace=bool(os.environ.get("NMS_TRACE")))
    LAST_RESULTS = res

    masks = np.stack([res.results[i]["mask"] for i in range(N_CORES)])
    mask_full = masks.reshape(B, C, H, W) != 0
    mask_full[:, :, :10, :] = False
    mask_full[:, :, -10:, :] = False
    mask_full[:, :, :, :10] = False
    mask_full[:, :, :, -10:] = False
    _, _, ys, xs = np.nonzero(mask_full)
    return np.stack([ys, xs]).astype(np.int32)


# revision 10
# speedup vs baseline: 1.3089x; 1.0190x over previous
"""3x3 NMS (maxpool + threshold + border) kernel for Trainium2, 8 NeuronCores.

Strategy:
  - Pure data parallel: 16 images -> 2 images per core on 8 cores.
  - Host zero-pads each image to H+2 rows so the kernel can load, per
    partition, R+2 consecutive rows (R=12 core rows + 1 halo row each
    side) with a single overlapping strided DMA. Partition p of a tile
    holds padded rows p*R .. p*R+R+1 (= image rows p*R-1 .. p*R+R).
  - The image is split into NT column tiles (2-col halos) to fit SBUF.
  - Work is split across two engines per tile:
      1. Pool (gpsimd): v1 = max(x_up, x_dn)   (vertical pair max)
      2. DVE: one fused custom uop (ANT_NMS_FUSED) that per stream
         element computes vm = max(v1, x), the horizontal sliding
         3-window max of vm, the 0.6 threshold clamp, and the compare
         mask[c] = (x[c] >= max(vm[c-1], vm[c], vm[c+1], 0.6)) in a
         single 1-elem/cycle pass, using delay-chain captures for the
         previous-element taps (including a 1-delayed x for the center
         compare).
    x >= max(window incl. center, thr) is exactly
    (x == maxpool3x3(x)) & (x >= thr): bit-exact, no FP hazards.
  - Host: zero 10px border, np.nonzero -> (y, x) rows, exactly matching
    jnp.nonzero order (batch-major, then row, then col).
"""

import os
import sys

sys.path.insert(0, "/opt/trn_rl_repo")

import numpy as np

B, C, H, W = 16, 1, 1536, 1536
HP = H + 2                    # padded rows
N_CORES = 8
B_PER = B // N_CORES          # images per core
R = 24                        # rows per partition (2 imgs * 64 blocks = 128)
NB = H // R                   # row blocks per image (64)
NT = 8                        # column tiles (both images per tile)
V = W // NT                   # valid (output) columns per tile
PAD = 2                       # column halo on each side
REP_THR = 0.6

_CACHE = {}
LAST_RESULTS = None


def _build_program():
    import concourse.bass as bass
    import concourse.bacc as bacc
    import concourse.mybir as mybir
    from concourse.tile import TileContext

    f32 = mybir.dt.float32
    u8 = mybir.dt.uint8
    MAX = mybir.AluOpType.max

    from concourse.dve_ops import DveOp, OPS, _COMPILE_CACHE
    from concourse.dve_spec import Spec, Src0, Src1, C0, maxx, lower
    from concourse.dve_uop import (
        DveOpSpec, InpSel, OutSel, OutPath, AluInp, DelayInp, AluOp,
    )
    from concourse.dve_ops import get_dve_sub_opcode


    def _mk_fused_uop(base_uop):
        """One fused NMS pass. Stream pos i carries v1[i] (src0) and x[i]
        (src1); output at pos i is mask for the column one behind:
        out(i) = (x(i-1) >= max(0.6, vm(i-2), vm(i-1), vm(i))) with
        vm(j) = max(v1(j), x(j)).

        Delay chains (v3 has 6): 0 = v1 in, 1 = C0, 2 = x in,
        3 = x delayed one element, 4 = vm(i-1) tap, 5 = m1(i-1) tap.
        """
        u = base_uop  # copy of a lowered stock uop: keeps FSM/trigger/ctrl
        for i in range(len(u.inp)):
            u.inp_enable[i] = 0
        u.enable_input(InpSel.SRC_0, 1)
        u.enable_input(InpSel.CONST_0, 2)
        u.enable_input(InpSel.SRC_1, 3)
        for p in u.out_enable:
            u.out_enable[p] = 0
        u.enable_output(OutSel.ALU_OUT, OutPath.WR0_LO)
        u.require_inp0 = 1
        u.require_inp1 = 1

        dp = u.datapath_config
        for b in dp:
            b.op = AluOp.BYPASS
            b.alu_src0 = AluInp.PREV_ALU_OUT
            b.alu_src1 = AluInp.PREV_ALU_OUT
            b.alu_out_enable = 1
            b.swap_enable = 0
            b.alu_out_a_enable = 0
            b.alu_out_b_enable = 0
            for c in range(len(b.delay)):
                b.delay[c] = DelayInp.PREV_ALU_OUT
                b.delay_enable[c] = 0

        # blk0: ALU = bypass(x); chain3 <- x (reads as x(i-1) downstream);
        #       carry v1 (0), C0 (1), x (2) onward
        dp[0].enable_alu(AluOp.BYPASS, AluInp.PREV_DELAY_2)
        dp[0].pass_through_delay(0, 1, 2)
        dp[0].enable_delay_from_src(DelayInp.CURR_ALU_OUT, 3)
        # blk1: vm = max(v1(i), x(i)); chain4 <- vm (reads as vm(i-1))
        dp[1].enable_alu(AluOp.MAX, AluInp.PREV_DELAY_0, AluInp.PREV_DELAY_2)
        dp[1].pass_through_delay(1, 3)
        dp[1].enable_delay_from_src(DelayInp.CURR_ALU_OUT, 4)
        # blk2: m1 = max(vm(i), vm(i-1)); chain5 <- m1 (reads as m1(i-1))
        dp[2].enable_alu(AluOp.MAX, AluInp.PREV_ALU_OUT, AluInp.PREV_DELAY_4)
        dp[2].pass_through_delay(1, 3)
        dp[2].enable_delay_from_src(DelayInp.CURR_ALU_OUT, 5)
        # blk3: M = max(m1(i), m1(i-1)) = max(vm(i-2..i))
        dp[3].enable_alu(AluOp.MAX, AluInp.PREV_ALU_OUT, AluInp.PREV_DELAY_5)
        dp[3].pass_through_delay(1, 3)
        # blk4: clamp with C0
        dp[4].enable_alu(AluOp.MAX, AluInp.PREV_ALU_OUT, AluInp.PREV_DELAY_1)
        dp[4].pass_through_delay(3)
        # blk5: out = (Mc <= x(i-1))  i.e. x(i-1) >= window max
        dp[5].enable_alu(AluOp.IS_LE, AluInp.PREV_ALU_OUT, AluInp.PREV_DELAY_3)
        return u


    _READY = {}


    def make_ops(ver="v3"):
        if _READY:
            return _READY["fused"]
        base = lower(Spec(body=maxx(maxx(Src0, C0), Src1)), ver=ver)
        assert len(base) == 1, len(base)

        fused_spec = Spec(body=maxx(maxx(Src0, C0), Src1))  # dummy; cache hit

        FUSED = DveOp("ANT_NMS_FUSED", fused_spec, subdim=False, uops_sha={})
        import concourse.dve_ops as dmod
        OPS.append(FUSED)
        for i, op in enumerate(OPS):
            dmod._SUB_OPCODE_FOR_NAME[op.name] = dmod._CUSTOM_DVE_ROW_BASE + i
        dmod.CUSTOM_DVE_SPECS[FUSED.name] = FUSED.spec

        uf = _mk_fused_uop(base[0])

        _COMPILE_CACHE[("ANT_NMS_FUSED", ver)] = DveOpSpec(
            name="ANT_NMS_FUSED", opcode=get_dve_sub_opcode("ANT_NMS_FUSED"),
            uops=[uf], rd1_en=True)
        _READY["fused"] = FUSED
        return FUSED

    FUSED = make_ops()

    nc = bacc.Bacc()
    # x: host-staged overlapping row blocks; partition p = img*NB + blk
    # holds rows blk*R-1 .. blk*R+R of image img (zero-padded at the edges).
    x_in = nc.declare_dram_parameter("x", [128, R + 2, W], f32, isOutput=False)
    # mask: [partition, row-in-block, col]; reshapes to [B_PER, H, W] on host.
    m_out = nc.declare_dram_parameter("mask", [128, R, W], u8, isOutput=True)

    with TileContext(nc) as tc:
        with tc.tile_pool(name="pool", bufs=1) as pool:
            for t in range(NT):
                cs = max(t * V - PAD, 0)
                ce = min(t * V + V + PAD, W)
                WT = ce - cs
                a = t * V - cs  # local col offset of the valid range

                xi = bass.AP(x_in, cs,
                             [[(R + 2) * W, 128], [W, R + 2], [1, WT]])

                X = pool.tile([128, R + 2, WT], f32, tag="X", bufs=3,
                              name=f"X_{t}")
                V1 = pool.tile([128, R, WT], f32, tag="V1", bufs=2,
                               name=f"V1_{t}")
                MSK = pool.tile([128, R, V + 2], u8, tag="MSK", bufs=2,
                                name=f"MSK_{t}")

                nc.sync.dma_start(out=X[:, :, :], in_=xi)

                # Vertical pair max of the two outer rows. (The gpsimd Pool
                # engine cannot run TT max in this toolchain: walrus codegen
                # only accepts Add/Multiply there.)
                nc.vector.tensor_tensor(
                    V1[:, :, :], X[:, 0:R, :], X[:, 2:R + 2, :], MAX)

                # DVE: fused merge + horizontal sliding max3 + clamp +
                # compare, row-major streams. Junk in the first 2 cols of
                # each row lands in discarded scratch cols (or border
                # cols 0,1 for the first tile).
                if t == 0:
                    # out col k = mask col k; window centered k
                    nc.vector._custom_dve(
                        FUSED,
                        out=MSK[:, :, 0:V],
                        in0=V1[:, :, 1:V + 1],
                        in1=X[:, 1:R + 1, 1:V + 1],
                        s0=REP_THR)
                    mo = bass.AP(m_out, 0,
                                 [[R * W, 128], [W, R], [1, V]])
                    nc.sync.dma_start(out=mo, in_=MSK[:, :, 0:V])
                else:
                    # out col k = mask col t*V-2+k; valid k in [2, V+2).
                    # On the last tile the final column's window would read
                    # past the image edge: shorten the stream by one and
                    # leave mask col W-1 (border, host-zeroed) junk.
                    SL = V + 2 if t < NT - 1 else V + 1
                    nc.vector._custom_dve(
                        FUSED,
                        out=MSK[:, :, 0:SL],
                        in0=V1[:, :, a - 1:a - 1 + SL],
                        in1=X[:, 1:R + 1, a - 1:a - 1 + SL],
                        s0=REP_THR)
                    mo = bass.AP(m_out, t * V,
                                 [[R * W, 128], [W, R], [1, V]])
                    nc.sync.dma_start(out=mo, in_=MSK[:, :, 2:V + 2])
    nc.finalize()
    return nc


def _get_program():
    if "nc" not in _CACHE:
        _CACHE["nc"] = _build_program()
    return _CACHE["nc"]


def kernel(repeatability):
    global LAST_RESULTS
    from concourse.bass_utils import run_bass_kernel_spmd

    x = np.asarray(repeatability, dtype=np.float32).reshape(B, H, W)
    xp = np.zeros((B, HP, W), dtype=np.float32)
    xp[:, 1:H + 1, :] = x
    # overlapping row blocks: [B, NB, R+2, W]; block b covers padded rows
    # b*R .. b*R+R+1 (= image rows b*R-1 .. b*R+R)
    st = xp.strides
    xb = np.lib.stride_tricks.as_strided(
        xp, shape=(B, NB, R + 2, W), strides=(st[0], R * st[1], st[1], st[2]))
    per_core = np.ascontiguousarray(
        xb.reshape(N_CORES, B_PER * NB, R + 2, W))
    in_maps = [{"x": per_core[i]} for i in range(N_CORES)]

    nc = _get_program()
    res = run_bass_kernel_spmd(nc, in_maps, list(range(N_CORES)),
                               trace=bool(os.environ.get("NMS_TRACE")))
    LAST_RESULTS = res

    masks = np.stack([res.results[i]["mask"] for i in range(N_CORES)])
    mask_full = masks.reshape(B, C, H, W) != 0
    mask_full[:, :, :10, :] = False
    mask_full[:, :, -10:, :] = False
    mask_full[:, :, :, :10] = False
    mask_full[:, :, :, -10:] = False
    _, _, ys, xs = np.nonzero(mask_full)
    return np.stack([ys, xs]).astype(np.int32)


# revision 11
# speedup vs baseline: 1.3942x; 1.0652x over previous
"""3x3 NMS (maxpool + threshold + border) kernel for Trainium2, 8 NeuronCores.

Strategy:
  - Pure data parallel: 16 images -> 2 images per core on 8 cores.
  - Both images of a core are packed into the partition dim: partition
    p = img*64 + blk holds R=24 image rows (+1 halo row each side, from
    a host-zero-padded copy), so the row-halo DMA overhead is 26/24.
  - The image is split into NT column tiles (2-col halos). The first and
    last tiles are narrow to shrink the pipeline fill / drain on the
    critical path.
  - Host stages the input TILE-MAJOR: for each tile, a contiguous
    [128, 26, WT] block (halo rows/cols duplicated), so every DMA
    descriptor is one full partition worth (26*WT*4 B) and the 16 DMA
    engines run at full rate. The mask output is likewise tile-major
    [128, 24, V+2] contiguous; host reassembles/strips junk columns.
  - Per tile, 2 vector-engine passes:
      1. v1 = max(x_up, x_dn)            (stock tensor_tensor, 1 el/cyc)
      2. mask = ANT_NMS_FUSED(v1, x): a custom DVE uop computing
         vm = max(v1, x), the horizontal sliding 3-max of vm, the 0.6
         clamp, and the compare  mask[c] = (x[c] >= max(vm[c-1..c+1],
         0.6))  in ONE 1-elem/cycle pass, using delay-chain captures for
         the previous-element taps (incl. a 1-delayed x for the center).
    x >= max(window incl. center, thr) is exactly
    (x == maxpool3x3(x)) & (x >= thr): bit-exact, no FP hazards.
  - Host: zero 10px border, np.nonzero -> (y, x) rows, exactly matching
    jnp.nonzero order (batch-major, then row, then col).
"""

import os
import sys

sys.path.insert(0, "/opt/trn_rl_repo")

import numpy as np

B, C, H, W = 16, 1, 1536, 1536
HP = H + 2                    # padded rows
N_CORES = 8
B_PER = B // N_CORES          # images per core
R = 24                        # rows per partition (2 imgs * 64 blocks = 128)
NB = H // R                   # row blocks per image (64)
PAD = 2                       # column halo on each side
REP_THR = 0.6

# column tile widths: narrow first/last for small pipeline fill/drain
WIDTHS = [64] + [224] * 6 + [128]
assert sum(WIDTHS) == W
NT = len(WIDTHS)
# tile t covers mask cols [C0[t], C0[t]+WIDTHS[t]), reads [cs, ce)
C0 = [sum(WIDTHS[:i]) for i in range(NT)]

_CACHE = {}
LAST_RESULTS = None


def _tile_geom(t):
    c0, v = C0[t], WIDTHS[t]
    cs = max(c0 - PAD, 0)
    ce = min(c0 + v + PAD, W)
    return c0, v, cs, ce - cs


def _build_program():
    import concourse.bass as bass
    import concourse.bacc as bacc
    import concourse.mybir as mybir
    from concourse.tile import TileContext

    f32 = mybir.dt.float32
    u8 = mybir.dt.uint8
    MAX = mybir.AluOpType.max

    from concourse.dve_ops import DveOp, OPS, _COMPILE_CACHE
    from concourse.dve_spec import Spec, Src0, Src1, C0 as DC0, maxx, lower
    from concourse.dve_uop import (
        DveOpSpec, InpSel, OutSel, OutPath, AluInp, DelayInp, AluOp,
    )
    from concourse.dve_ops import get_dve_sub_opcode


    def _mk_fused_uop(base_uop):
        """One fused NMS pass. Stream pos i carries v1[i] (src0) and x[i]
        (src1); output at pos i is mask for the column one behind:
        out(i) = (x(i-1) >= max(0.6, vm(i-2), vm(i-1), vm(i))) with
        vm(j) = max(v1(j), x(j)).

        Delay chains (v3 has 6): 0 = v1 in, 1 = C0, 2 = x in,
        3 = x delayed one element, 4 = vm(i-1) tap, 5 = m1(i-1) tap.
        """
        u = base_uop  # copy of a lowered stock uop: keeps FSM/trigger/ctrl
        for i in range(len(u.inp)):
            u.inp_enable[i] = 0
        u.enable_input(InpSel.SRC_0, 1)
        u.enable_input(InpSel.CONST_0, 2)
        u.enable_input(InpSel.SRC_1, 3)
        for p in u.out_enable:
            u.out_enable[p] = 0
        u.enable_output(OutSel.ALU_OUT, OutPath.WR0_LO)
        u.require_inp0 = 1
        u.require_inp1 = 1

        dp = u.datapath_config
        for b in dp:
            b.op = AluOp.BYPASS
            b.alu_src0 = AluInp.PREV_ALU_OUT
            b.alu_src1 = AluInp.PREV_ALU_OUT
            b.alu_out_enable = 1
            b.swap_enable = 0
            b.alu_out_a_enable = 0
            b.alu_out_b_enable = 0
            for c in range(len(b.delay)):
                b.delay[c] = DelayInp.PREV_ALU_OUT
                b.delay_enable[c] = 0

        # blk0: ALU = bypass(x); chain3 <- x (reads as x(i-1) downstream);
        #       carry v1 (0), C0 (1), x (2) onward
        dp[0].enable_alu(AluOp.BYPASS, AluInp.PREV_DELAY_2)
        dp[0].pass_through_delay(0, 1, 2)
        dp[0].enable_delay_from_src(DelayInp.CURR_ALU_OUT, 3)
        # blk1: vm = max(v1(i), x(i)); chain4 <- vm (reads as vm(i-1))
        dp[1].enable_alu(AluOp.MAX, AluInp.PREV_DELAY_0, AluInp.PREV_DELAY_2)
        dp[1].pass_through_delay(1, 3)
        dp[1].enable_delay_from_src(DelayInp.CURR_ALU_OUT, 4)
        # blk2: m1 = max(vm(i), vm(i-1)); chain5 <- m1 (reads as m1(i-1))
        dp[2].enable_alu(AluOp.MAX, AluInp.PREV_ALU_OUT, AluInp.PREV_DELAY_4)
        dp[2].pass_through_delay(1, 3)
        dp[2].enable_delay_from_src(DelayInp.CURR_ALU_OUT, 5)
        # blk3: M = max(m1(i), m1(i-1)) = max(vm(i-2..i))
        dp[3].enable_alu(AluOp.MAX, AluInp.PREV_ALU_OUT, AluInp.PREV_DELAY_5)
        dp[3].pass_through_delay(1, 3)
        # blk4: clamp with C0
        dp[4].enable_alu(AluOp.MAX, AluInp.PREV_ALU_OUT, AluInp.PREV_DELAY_1)
        dp[4].pass_through_delay(3)
        # blk5: out = (Mc <= x(i-1))  i.e. x(i-1) >= window max
        dp[5].enable_alu(AluOp.IS_LE, AluInp.PREV_ALU_OUT, AluInp.PREV_DELAY_3)
        return u


    _READY = {}


    def make_ops(ver="v3"):
        if _READY:
            return _READY["fused"]
        base = lower(Spec(body=maxx(maxx(Src0, DC0), Src1)), ver=ver)
        assert len(base) == 1, len(base)

        fused_spec = Spec(body=maxx(maxx(Src0, DC0), Src1))  # dummy; cache hit

        FUSED = DveOp("ANT_NMS_FUSED", fused_spec, subdim=False, uops_sha={})
        import concourse.dve_ops as dmod
        OPS.append(FUSED)
        for i, op in enumerate(OPS):
            dmod._SUB_OPCODE_FOR_NAME[op.name] = dmod._CUSTOM_DVE_ROW_BASE + i
        dmod.CUSTOM_DVE_SPECS[FUSED.name] = FUSED.spec

        uf = _mk_fused_uop(base[0])

        _COMPILE_CACHE[("ANT_NMS_FUSED", ver)] = DveOpSpec(
            name="ANT_NMS_FUSED", opcode=get_dve_sub_opcode("ANT_NMS_FUSED"),
            uops=[uf], rd1_en=True)
        _READY["fused"] = FUSED
        return FUSED

    FUSED = make_ops()

    # tile-major staged input: for tile t a contiguous [128, 26, WT] block
    XTOT = sum(_tile_geom(t)[3] for t in range(NT)) * (R + 2) * 128
    # tile-major mask out: for tile t a contiguous [128, 24, V+2] block
    MSKW = [WIDTHS[t] + 2 for t in range(NT)]
    MTOT = sum(MSKW) * R * 128

    nc = bacc.Bacc()
    x_in = nc.declare_dram_parameter("x", [XTOT], f32, isOutput=False)
    m_out = nc.declare_dram_parameter("mask", [MTOT], u8, isOutput=True)

    with TileContext(nc) as tc:
        with tc.tile_pool(name="pool", bufs=1) as pool:
            xoff = 0
            moff = 0
            for t in range(NT):
                c0, v, cs, WT = _tile_geom(t)
                a = c0 - cs  # local col offset of the valid range
                WM = MSKW[t]

                xi = bass.AP(x_in, xoff,
                             [[(R + 2) * WT, 128], [WT, R + 2], [1, WT]])
                xoff += 128 * (R + 2) * WT

                X = pool.tile([128, R + 2, WT], f32, tag="X", bufs=3,
                              name=f"X_{t}")
                V1 = pool.tile([128, R, WT], f32, tag="V1", bufs=2,
                               name=f"V1_{t}")
                MSK = pool.tile([128, R, WM], u8, tag="MSK", bufs=2,
                                name=f"MSK_{t}")

                nc.sync.dma_start(out=X[:, :, :], in_=xi)

                # Vertical pair max of the two outer rows. (The gpsimd Pool
                # engine cannot run TT max in this toolchain: walrus codegen
                # only accepts Add/Multiply there.)
                nc.vector.tensor_tensor(
                    V1[:, :, :], X[:, 0:R, :], X[:, 2:R + 2, :], MAX)

                # DVE: fused merge + horizontal sliding max3 + clamp +
                # compare, row-major streams. Junk in the first 2 cols of
                # each row lands in discarded scratch cols (or border
                # cols 0,1 for the first tile).
                if t == 0:
                    # out col k = mask col k; window centered k. MSK cols
                    # v..v+1 stay junk; host reads [0:v].
                    nc.vector._custom_dve(
                        FUSED,
                        out=MSK[:, :, 0:v],
                        in0=V1[:, :, 1:v + 1],
                        in1=X[:, 1:R + 1, 1:v + 1],
                        s0=REP_THR)
                else:
                    # out col k = mask col c0-2+k; valid k in [2, v+2); host
                    # reads [2:v+2]. On the last tile the final column's
                    # window would read past the image edge: shorten the
                    # stream by one; mask col W-1 junk is border, host-zeroed.
                    SL = v + 2 if t < NT - 1 else v + 1
                    nc.vector._custom_dve(
                        FUSED,
                        out=MSK[:, :, 0:SL],
                        in0=V1[:, :, a - 1:a - 1 + SL],
                        in1=X[:, 1:R + 1, a - 1:a - 1 + SL],
                        s0=REP_THR)
                mo = bass.AP(m_out, moff, [[R * WM, 128], [1, R * WM]])
                moff += 128 * R * WM
                nc.sync.dma_start(out=mo, in_=MSK[:, :, :])
    nc.finalize()
    return nc


def _get_program():
    if "nc" not in _CACHE:
        _CACHE["nc"] = _build_program()
    return _CACHE["nc"]


def kernel(repeatability):
    global LAST_RESULTS
    from concourse.bass_utils import run_bass_kernel_spmd

    x = np.asarray(repeatability, dtype=np.float32).reshape(B, H, W)
    xp = np.zeros((B, HP, W), dtype=np.float32)
    xp[:, 1:H + 1, :] = x
    # overlapping row blocks: [B, NB, R+2, W]; block b covers padded rows
    # b*R .. b*R+R+1 (= image rows b*R-1 .. b*R+R)
    st = xp.strides
    xb = np.lib.stride_tricks.as_strided(
        xp, shape=(B, NB, R + 2, W), strides=(st[0], R * st[1], st[1], st[2]))
    xb = xb.reshape(N_CORES, B_PER * NB, R + 2, W)

    # stage tile-major: per core, concat per-tile [128, 26, WT] blocks
    in_maps = []
    for i in range(N_CORES):
        parts = []
        for t in range(NT):
            _, _, cs, WT = _tile_geom(t)
            parts.append(
                np.ascontiguousarray(xb[i, :, :, cs:cs + WT]).reshape(-1))
        in_maps.append({"x": np.concatenate(parts)})

    nc = _get_program()
    res = run_bass_kernel_spmd(nc, in_maps, list(range(N_CORES)),
                               trace=bool(os.environ.get("NMS_TRACE")))
    LAST_RESULTS = res

    # reassemble masks: per tile t the block is [128, 24, V+2]; valid cols
    # are [0:v] for t=0 else [2:v+2]
    mask_full = np.empty((N_CORES, 128, R, W), dtype=np.uint8)
    for i in range(N_CORES):
        flat = res.results[i]["mask"]
        off = 0
        for t in range(NT):
            c0, v = C0[t], WIDTHS[t]
            wm = v + 2
            blk = flat[off:off + 128 * R * wm].reshape(128, R, wm)
            off += 128 * R * wm
            sl = blk[:, :, 0:v] if t == 0 else blk[:, :, 2:v + 2]
            mask_full[i, :, :, c0:c0 + v] = sl
    mask_full = mask_full.reshape(B, C, H, W) != 0
    mask_full[:, :, :10, :] = False
    mask_full[:, :, -10:, :] = False
    mask_full[:, :, :, :10] = False
    mask_full[:, :, :, -10:] = False
    _, _, ys, xs = np.nonzero(mask_full)
    return np.stack([ys, xs]).astype(np.int32)
